# revision 19
# baseline (speedup 1.0000x reference)
"""Conv2d-via-FFT reference implemented as a direct convolution on TRN2.

The reference pads to FFT size 61 >= 32+3-1, so its circular cross-correlation
equals the linear valid cross-correlation: out[n,f,i,j] =
sum_{c,p,q} x[n,c,i+p,j+q] * w[f,c,p,q] + bias[f].  That is an ordinary
stride-1 valid conv2d, which maps onto the PE array as 9 accumulated matmuls
(one per filter tap) with C=128 on the contraction partitions, float32r
operands (full-rate fp32 path, ~1.3e-4 rel err), fp32 PSUM accumulation.

Sharding: data-parallel over N (64 samples -> 8 per core), filter replicated.

Raw bass (no Tile scheduler): 5 semaphores, hand-placed waits.  Per core:
  Sync    engine: 16 x-input DMA triggers (sample halves, double-buffered x3)
  Scalar  engine: bias + 9 w-tap DMAs, then per chunk ACTIVATE(+bias) + out DMA
  Tensor  engine: 16 chunks x 9 accumulated matmuls [128c x 128f x 450px]
"""

import numpy as np

import concourse.bass as bass
import concourse.bacc as bacc
import concourse.mybir as mybir
from concourse.bass_utils import run_bass_kernel_spmd

dt = mybir.dt
F32 = dt.float32
F32R = dt.float32r
IDENT = mybir.ActivationFunctionType.Identity

N, C, H, W = 64, 128, 32, 32
F, KH, KW = 128, 3, 3
KK = KH * KW
OH, OW = H - KH + 1, W - KW + 1          # 30, 30
NCORES = 8
NPC = N // NCORES                        # samples per core
RPC = 15                                 # rows per chunk -> 450 px per matmul
NCHUNK = OH // RPC
CPX = RPC * OW                           # 450 <= 512 (one PSUM bank)
NC_CHUNKS = NPC * NCHUNK                 # 16 chunks per core
OBUF, PSBUF = 4, 4
NWARM = 7                                # HAM warmup matmuls
# x-load plan: sample 0 streams in three row pieces (compute starts after the
# first 7 rows); the rest arrive as paired-sample DMAs whose 8KB-per-partition
# descriptors run the SDMA engines near peak bandwidth.
X0_PIECES = [(0, 7), (7, 17), (17, 32)]
X_GROUPS = [(1, 3), (3, 5), (5, 7), (7, 8)]          # [lo, hi) sample ranges

# Per-sample chunk layout (row0, nrows): sample 0 front-loads a small chunk so
# real matmuls start as soon as the first few x rows land; the last sample
# ends with a small chunk so the final ACT+store drains quickly.
def _sample_chunks(n):
    if n == 0:
        return [(0, 5), (5, 10), (15, 15)]
    if n == NPC - 1:
        return [(0, 15), (15, 10), (25, 5)]
    return [(0, 15), (15, 15)]

# Per-sample x-load row pieces (lo, hi): prefixes cover each chunk's rows.
CHUNKS = [(n, row0, nrows) for n in range(NPC) for row0, nrows in _sample_chunks(n)]
NFLAT = len(CHUNKS)

def _x0_pieces_needed(row0, nrows):
    hi = row0 + nrows + KH - 2               # last x row read (inclusive)
    for pi, (lo, phi) in enumerate(X0_PIECES):
        if hi < phi:
            return pi + 1
    raise AssertionError

def _group_of(n):
    return next(g for g, (lo, hi) in enumerate(X_GROUPS) if lo <= n < hi)


def _build():
    nc = bacc.Bacc("TRN2", target_bir_lowering=False, debug=False)

    x_d = nc.dram_tensor("x", [C, NPC, H, W], F32R, kind="ExternalInput").ap()
    w_d = nc.dram_tensor("w", [C, KK, F], F32R, kind="ExternalInput").ap()
    b_d = nc.dram_tensor("bias", [F, 1], F32, kind="ExternalInput").ap()
    o_d = nc.dram_tensor("out", [NPC, F, OH * OW], F32, kind="ExternalOutput").ap()

    w_sb = nc.alloc_sbuf_tensor("w_sb", [C, KK, F], F32R).ap()
    b_sb = nc.alloc_sbuf_tensor("b_sb", [F, 1], F32).ap()
    x_sb = nc.alloc_sbuf_tensor("x_sb", [C, NPC, H, W], F32R).ap()
    o_sb = [nc.alloc_sbuf_tensor(f"o_sb{i}", [F, CPX], F32).ap()
            for i in range(OBUF)]
    ps = [nc.alloc_psum_tensor(f"ps{i}", [F, CPX], F32).ap()
          for i in range(PSBUF)]
    ps_warm = nc.alloc_psum_tensor("ps_warm", [F, 512], F32).ap()

    # HWDGE semantics: a DMA's +16 arrives as 16 independent +1s (one per SDMA
    # engine), so a wait at an intermediate threshold on a sem with a second
    # DMA in flight can pass on mixed partial completions.  Sound pattern:
    # dedicate a sem per buffer slot and only ever wait for the maximum value
    # possible at that point (all DMAs issued on that sem so far complete).
    # Sem numbers are pinned into 207..: the NEFF epilogue blanket-resets all
    # 249 kernel sems split per engine (~50 each, ~115ns/sem), and the Sync
    # engine owns the 207..255 slice (the only slice inside the bass-visible
    # 155..255 pool whose owner we can make finish last).  Sync gates on the
    # all-outputs-landed waits, so its reset of live sems is ordered after
    # completion, no exit barrier is needed, and the other engines' reset
    # storms overlap compute.
    from contextlib import ExitStack
    with ExitStack() as ctx:
      _next_num = iter(range(207, 250))
      sem = lambda nm: ctx.enter_context(nc.semaphore(nm, num=next(_next_num)))
      s_x0 = [sem(f"s_x0_{p}") for p in range(len(X0_PIECES))]
      s_xg = [sem(f"s_xg{g}") for g in range(len(X_GROUPS))]
      s_wg = [sem(f"s_wg{g}") for g in range(3)]      # w tap groups of 3
      s_b = sem("s_b")
      s_o = [sem(f"s_o{j}") for j in range(OBUF)]     # out DMA per o_sb slot
      s_mm = sem("s_mm")
      s_act = sem("s_act")

      _orig_barrier = nc.all_engine_barrier
      nc.all_engine_barrier = lambda *a, **k: None
      with nc.Block(no_gpsimd_drain=True) as block:

        @block.sync
        def _(sync):
            # w group 0 ahead of all x traffic: it is the first LDW dependency
            sync.dma_start(w_sb[:, 0:3], w_d[:, 0:3]).then_inc(s_wg[0], 16)
            for p, (lo, hi) in enumerate(X0_PIECES):
                sync.dma_start(x_sb[:, 0, lo:hi],
                               x_d[:, 0, lo:hi]).then_inc(s_x0[p], 16)
            for g, (lo, hi) in enumerate(X_GROUPS):
                sync.dma_start(x_sb[:, lo:hi],
                               x_d[:, lo:hi]).then_inc(s_xg[g], 16)
            for j in range(OBUF):                     # all outputs in DRAM
                sync.wait_ge(s_o[j], 16 * ((NFLAT + OBUF - 1 - j) // OBUF))

        @block.scalar
        def _(scalar):
            scalar.dma_start(b_sb[:], b_d[:]).then_inc(s_b, 16)
            for g in range(1, 3):
                scalar.dma_start(w_sb[:, 3 * g:3 * g + 3],
                                 w_d[:, 3 * g:3 * g + 3]).then_inc(s_wg[g], 16)
            for i, (n, row0, nrows) in enumerate(CHUNKS):
                px = nrows * OW
                if i >= OBUF:
                    # o_sb slot free once its previous out DMA fully drained
                    scalar.wait_ge(s_o[i % OBUF], 16 * (i // OBUF))
                if i == 0:
                    scalar.wait_ge(s_b, 16)           # bias landed
                scalar.wait_ge(s_mm, i + 1)           # chunk accumulated
                nc.scalar.activation(o_sb[i % OBUF][:, :px], ps[i % PSBUF][:, :px],
                                     IDENT, bias=b_sb[:]).then_inc(s_act, 1)
                scalar.dma_start(o_d[n, :, row0 * OW:row0 * OW + px],
                                 o_sb[i % OBUF][:, :px]).then_inc(s_o[i % OBUF], 16)

        @block.tensor
        def _(tensor):
            # No-dependency warmup matmuls on whatever is in SBUF: keeps the
            # PE busy from kernel entry so the HAM clock gate opens (K=8/8)
            # before the real matmuls start.  Results land in a scratch bank.
            for _ in range(NWARM):
                nc.tensor.matmul(ps_warm[:], w_sb[:, 0], x_sb[:, 0, 0:16, :],
                                 start=True, stop=True)
            for i, (n, row0, nrows) in enumerate(CHUNKS):
                if i >= PSBUF:
                    tensor.wait_ge(s_act, i - PSBUF + 1)   # bank drained
                if i == 0:
                    tensor.wait_ge(s_wg[0], 16)
                for k in range(KK):
                    p, q = divmod(k, KW)
                    mm = nc.tensor.matmul(
                        ps[i % PSBUF][:, :nrows * OW],
                        w_sb[:, k],
                        x_sb[:, n, row0 + p:row0 + p + nrows, q:q + OW],
                        start=(k == 0),
                        stop=(k == KK - 1),
                    )
                    if k == 0:
                        # sample 0: pieces are FIFO on one ring set, so the
                        # last needed piece's completion implies the earlier
                        # ones; other samples wait on their group DMA
                        if n == 0:
                            mm._wait_ge(s_x0[_x0_pieces_needed(row0, nrows) - 1], 16)
                        else:
                            mm._wait_ge(s_xg[_group_of(n)], 16)
                    if i == 0 and k in (3, 6):
                        mm._wait_ge(s_wg[k // 3], 16)      # tap group landed
                    if k == KK - 1:
                        mm.then_inc(s_mm, 1)

      nc.all_engine_barrier = _orig_barrier
      # No explicit sem clear needed: the NEFF epilogue's blanket per-engine
      # reset zeroes every kernel sem, and all increments have retired by the
      # time the Scalar engine (owner of 54..104) reaches its resets.

    nc.compile()
    return nc


_NC = None


def _get_nc():
    global _NC
    if _NC is None:
        _NC = _build()
    return _NC


def _in_maps(x, w, bias):
    w_prep = np.ascontiguousarray(
        w.transpose(1, 2, 3, 0).reshape(C, KK, F).astype(np.float32))
    b_prep = np.ascontiguousarray(bias.astype(np.float32).reshape(F, 1))
    maps = []
    for c in range(NCORES):
        xc = np.ascontiguousarray(
            x[c * NPC:(c + 1) * NPC].transpose(1, 0, 2, 3).astype(np.float32))
        maps.append({"x": xc, "w": w_prep, "bias": b_prep})
    return maps


def run(x, w, bias, trace=False, **spmd_kwargs):
    """Run the SPMD kernel; returns (out [N,F,OH,OW], BassKernelResults)."""
    nc = _get_nc()
    res = run_bass_kernel_spmd(nc, _in_maps(x, w, bias), list(range(NCORES)),
                               trace=trace, **spmd_kwargs)
    parts = [res.results[c]["out"].reshape(NPC, F, OH, OW) for c in range(NCORES)]
    return np.concatenate(parts, axis=0), res


def kernel(x, w, bias):
    out, _ = run(np.asarray(x), np.asarray(w), np.asarray(bias))
    return out


# revision 20
# speedup vs baseline: 1.0257x; 1.0257x over previous
"""Conv2d-via-FFT reference implemented as a direct convolution on TRN2.

The reference pads to FFT size 61 >= 32+3-1, so its circular cross-correlation
equals the linear valid cross-correlation: out[n,f,i,j] =
sum_{c,p,q} x[n,c,i+p,j+q] * w[f,c,p,q] + bias[f].  That is an ordinary
stride-1 valid conv2d, which maps onto the PE array as 9 accumulated matmuls
(one per filter tap) with C=128 on the contraction partitions, float32r
operands (full-rate fp32 path, ~1.3e-4 rel err), fp32 PSUM accumulation.

Sharding: data-parallel over N (64 samples -> 8 per core), filter replicated.

Raw bass (no Tile scheduler): 5 semaphores, hand-placed waits.  Per core:
  Sync    engine: 16 x-input DMA triggers (sample halves, double-buffered x3)
  Scalar  engine: bias + 9 w-tap DMAs, then per chunk ACTIVATE(+bias) + out DMA
  Tensor  engine: 16 chunks x 9 accumulated matmuls [128c x 128f x 450px]
"""

import numpy as np

import concourse.bass as bass
import concourse.bacc as bacc
import concourse.mybir as mybir
from concourse.bass_utils import run_bass_kernel_spmd

dt = mybir.dt
F32 = dt.float32
F32R = dt.float32r
IDENT = mybir.ActivationFunctionType.Identity

N, C, H, W = 64, 128, 32, 32
F, KH, KW = 128, 3, 3
KK = KH * KW
OH, OW = H - KH + 1, W - KW + 1          # 30, 30
NCORES = 8
NPC = N // NCORES                        # samples per core
RPC = 15                                 # rows per chunk -> 450 px per matmul
NCHUNK = OH // RPC
CPX = RPC * OW                           # 450 <= 512 (one PSUM bank)
NC_CHUNKS = NPC * NCHUNK                 # 16 chunks per core
OBUF, PSBUF = 4, 4
NWARM = 7                                # HAM warmup matmuls
# x-load plan: sample 0 streams in three row pieces (compute starts after the
# first 7 rows); the rest arrive as paired-sample DMAs whose 8KB-per-partition
# descriptors run the SDMA engines near peak bandwidth.
X0_PIECES = [(0, 7), (7, 17), (17, 32)]
# samples 1-2 stream in row halves (lower latency to first chunk); samples
# 3-7 as whole-sample DMAs (4KB-per-partition descriptors, better bandwidth)
X_HALVED = (1, 2)

# Per-sample chunk layout (row0, nrows): sample 0 front-loads a small chunk so
# real matmuls start as soon as the first few x rows land; the last sample
# ends with a small chunk so the final ACT+store drains quickly.
def _sample_chunks(n):
    if n == 0:
        return [(0, 5), (5, 10), (15, 15)]
    if n == NPC - 1:
        return [(0, 15), (15, 10), (25, 5)]
    return [(0, 15), (15, 15)]

# Per-sample x-load row pieces (lo, hi): prefixes cover each chunk's rows.
CHUNKS = [(n, row0, nrows) for n in range(NPC) for row0, nrows in _sample_chunks(n)]
NFLAT = len(CHUNKS)

def _x0_pieces_needed(row0, nrows):
    hi = row0 + nrows + KH - 2               # last x row read (inclusive)
    for pi, (lo, phi) in enumerate(X0_PIECES):
        if hi < phi:
            return pi + 1
    raise AssertionError



def _build():
    nc = bacc.Bacc("TRN2", target_bir_lowering=False, debug=False)

    x_d = nc.dram_tensor("x", [C, NPC, H, W], F32R, kind="ExternalInput").ap()
    w_d = nc.dram_tensor("w", [C, KK, F], F32R, kind="ExternalInput").ap()
    b_d = nc.dram_tensor("bias", [F, 1], F32, kind="ExternalInput").ap()
    o_d = nc.dram_tensor("out", [NPC, F, OH * OW], F32, kind="ExternalOutput").ap()

    w_sb = nc.alloc_sbuf_tensor("w_sb", [C, KK, F], F32R).ap()
    b_sb = nc.alloc_sbuf_tensor("b_sb", [F, 1], F32).ap()
    x_sb = nc.alloc_sbuf_tensor("x_sb", [C, NPC, H, W], F32R).ap()
    o_sb = [nc.alloc_sbuf_tensor(f"o_sb{i}", [F, CPX], F32).ap()
            for i in range(OBUF)]
    ps = [nc.alloc_psum_tensor(f"ps{i}", [F, CPX], F32).ap()
          for i in range(PSBUF)]
    ps_warm = nc.alloc_psum_tensor("ps_warm", [F, 512], F32).ap()

    # HWDGE semantics: a DMA's +16 arrives as 16 independent +1s (one per SDMA
    # engine), so a wait at an intermediate threshold on a sem with a second
    # DMA in flight can pass on mixed partial completions.  Sound pattern:
    # dedicate a sem per buffer slot and only ever wait for the maximum value
    # possible at that point (all DMAs issued on that sem so far complete).
    # Sem numbers are pinned into 207..: the NEFF epilogue blanket-resets all
    # 249 kernel sems split per engine (~50 each, ~115ns/sem), and the Sync
    # engine owns the 207..255 slice (the only slice inside the bass-visible
    # 155..255 pool whose owner we can make finish last).  Sync gates on the
    # all-outputs-landed waits, so its reset of live sems is ordered after
    # completion, no exit barrier is needed, and the other engines' reset
    # storms overlap compute.
    from contextlib import ExitStack
    with ExitStack() as ctx:
      _next_num = iter(range(207, 250))
      sem = lambda nm: ctx.enter_context(nc.semaphore(nm, num=next(_next_num)))
      s_x0 = [sem(f"s_x0_{p}") for p in range(len(X0_PIECES))]
      s_xh = {n: (sem(f"s_x{n}a"), sem(f"s_x{n}b")) for n in X_HALVED}
      s_xs = {n: sem(f"s_xs{n}") for n in range(1, NPC) if n not in X_HALVED}
      s_wg = [sem(f"s_wg{g}") for g in range(3)]      # w tap groups of 3
      s_b = sem("s_b")
      s_o = [sem(f"s_o{j}") for j in range(OBUF)]     # out DMA per o_sb slot
      s_mm = sem("s_mm")
      s_act = sem("s_act")

      _orig_barrier = nc.all_engine_barrier
      nc.all_engine_barrier = lambda *a, **k: None
      with nc.Block(no_gpsimd_drain=True) as block:

        @block.sync
        def _(sync):
            # w group 0 ahead of all x traffic: it is the first LDW dependency
            sync.dma_start(w_sb[:, 0:3], w_d[:, 0:3]).then_inc(s_wg[0], 16)
            for p, (lo, hi) in enumerate(X0_PIECES):
                sync.dma_start(x_sb[:, 0, lo:hi],
                               x_d[:, 0, lo:hi]).then_inc(s_x0[p], 16)
            for n in range(1, NPC):
                if n in X_HALVED:
                    sync.dma_start(x_sb[:, n, 0:17],
                                   x_d[:, n, 0:17]).then_inc(s_xh[n][0], 16)
                    sync.dma_start(x_sb[:, n, 17:32],
                                   x_d[:, n, 17:32]).then_inc(s_xh[n][1], 16)
                else:
                    sync.dma_start(x_sb[:, n],
                                   x_d[:, n]).then_inc(s_xs[n], 16)
            for j in range(OBUF):                     # all outputs in DRAM
                sync.wait_ge(s_o[j], 16 * ((NFLAT + OBUF - 1 - j) // OBUF))

        @block.scalar
        def _(scalar):
            scalar.dma_start(b_sb[:], b_d[:]).then_inc(s_b, 16)
            for g in range(1, 3):
                scalar.dma_start(w_sb[:, 3 * g:3 * g + 3],
                                 w_d[:, 3 * g:3 * g + 3]).then_inc(s_wg[g], 16)
            for i, (n, row0, nrows) in enumerate(CHUNKS):
                px = nrows * OW
                if i >= OBUF:
                    # o_sb slot free once its previous out DMA fully drained
                    scalar.wait_ge(s_o[i % OBUF], 16 * (i // OBUF))
                if i == 0:
                    scalar.wait_ge(s_b, 16)           # bias landed
                scalar.wait_ge(s_mm, i + 1)           # chunk accumulated
                nc.scalar.activation(o_sb[i % OBUF][:, :px], ps[i % PSBUF][:, :px],
                                     IDENT, bias=b_sb[:]).then_inc(s_act, 1)
                scalar.dma_start(o_d[n, :, row0 * OW:row0 * OW + px],
                                 o_sb[i % OBUF][:, :px]).then_inc(s_o[i % OBUF], 16)

        @block.tensor
        def _(tensor):
            # No-dependency warmup matmuls on whatever is in SBUF: keeps the
            # PE busy from kernel entry so the HAM clock gate opens (K=8/8)
            # before the real matmuls start.  Results land in a scratch bank.
            for _ in range(NWARM):
                nc.tensor.matmul(ps_warm[:], w_sb[:, 0], x_sb[:, 0, 0:16, :],
                                 start=True, stop=True)
            for i, (n, row0, nrows) in enumerate(CHUNKS):
                if i >= PSBUF:
                    tensor.wait_ge(s_act, i - PSBUF + 1)   # bank drained
                if i == 0:
                    tensor.wait_ge(s_wg[0], 16)
                for k in range(KK):
                    p, q = divmod(k, KW)
                    mm = nc.tensor.matmul(
                        ps[i % PSBUF][:, :nrows * OW],
                        w_sb[:, k],
                        x_sb[:, n, row0 + p:row0 + p + nrows, q:q + OW],
                        start=(k == 0),
                        stop=(k == KK - 1),
                    )
                    if k == 0:
                        # sample 0: pieces are FIFO on one ring set, so the
                        # last needed piece's completion implies the earlier
                        # ones; other samples wait on their group DMA
                        if n == 0:
                            mm._wait_ge(s_x0[_x0_pieces_needed(row0, nrows) - 1], 16)
                        elif n in X_HALVED:
                            # half B completion implies half A (same ring, FIFO)
                            hi_row = row0 + nrows + KH - 2
                            mm._wait_ge(s_xh[n][0 if hi_row < 17 else 1], 16)
                        else:
                            mm._wait_ge(s_xs[n], 16)
                    if i == 0 and k in (3, 6):
                        mm._wait_ge(s_wg[k // 3], 16)      # tap group landed
                    if k == KK - 1:
                        mm.then_inc(s_mm, 1)

      nc.all_engine_barrier = _orig_barrier
      # No explicit sem clear needed: the NEFF epilogue's blanket per-engine
      # reset zeroes every kernel sem, and all increments have retired by the
      # time the Scalar engine (owner of 54..104) reaches its resets.

    nc.compile()
    return nc


_NC = None


def _get_nc():
    global _NC
    if _NC is None:
        _NC = _build()
    return _NC


def _in_maps(x, w, bias):
    w_prep = np.ascontiguousarray(
        w.transpose(1, 2, 3, 0).reshape(C, KK, F).astype(np.float32))
    b_prep = np.ascontiguousarray(bias.astype(np.float32).reshape(F, 1))
    maps = []
    for c in range(NCORES):
        xc = np.ascontiguousarray(
            x[c * NPC:(c + 1) * NPC].transpose(1, 0, 2, 3).astype(np.float32))
        maps.append({"x": xc, "w": w_prep, "bias": b_prep})
    return maps


def run(x, w, bias, trace=False, **spmd_kwargs):
    """Run the SPMD kernel; returns (out [N,F,OH,OW], BassKernelResults)."""
    nc = _get_nc()
    res = run_bass_kernel_spmd(nc, _in_maps(x, w, bias), list(range(NCORES)),
                               trace=trace, **spmd_kwargs)
    parts = [res.results[c]["out"].reshape(NPC, F, OH, OW) for c in range(NCORES)]
    return np.concatenate(parts, axis=0), res


def kernel(x, w, bias):
    out, _ = run(np.asarray(x), np.asarray(w), np.asarray(bias))
    return out
